# revision 1
# baseline (speedup 1.0000x reference)
"""Trainium2 Bass kernel for a 3-layer dense-adjacency GCN decoder.

Problem (per batch graph): 3x GCN layer (msg = h@W + b; agg = A @ msg; relu)
followed by output projection + node mask. B=8 graphs of N=2048 nodes,
latent=64, hidden=128, out=64. Batch-parallel: one graph per NeuronCore.

v7 per-core plan (PE-bound kernel; agg matmuls stream 2 rows/cycle in bf16):
  - The host hands each core A^T cast to bf16 in a panel-contiguous layout
    ATP[i][p][j][c] = A^T[j*128+p, i*512+c].  The stream runs on a single
    queue in strict panel order (panel 0 in quarters, the rest in halves)
    so completion tracks consumption; all queues wake together at ~7.2us
    after the framework preamble.
  - Layer 0 is reassociated: A@(X@W0+b0) = (A@[X|1]) @ [[W0];[b0]].  The
    A-pass runs directly on the host-provided node-major [X|1] (bf16,
    stationary), chasing the panel stream; a 65-contract post-multiply
    with [[W0];[b0]] + fused ReLU produces hT1.  No transposes, no msg0
    stage.
  - msg chunks for layers 1-2 are produced node-major directly
    (stationary = hT feature slice, moving = W) — no PE transposes
    anywhere in the kernel; bias rides the PSUM evacuation as a
    host-broadcast [128,HID] tile via scalar_tensor_tensor, alternating
    DVE / gpsimd.
  - Layer-1 partial aggregations fill every PE wait: all steps runnable
    without panel i are emitted before chunk i's aggs (the PE queue is
    in-order), the rest interleave between chain links.
  - The final-panel drain is software-pipelined (chunk ip+1's aggs cover
    chunk ip's ReLU + msg2 production).
  - Output stays feature-major: proj chunk = mm + per-partition bias evac
    to bf16, DMA'd out as Y^T slices on rotating queues.  The host does
    the final transpose + node-mask multiply + f32 upcast (pure numpy on
    layout-sized data).  The last chunk runs in 256-halves so the
    trailing serial chain is short.
"""

import functools

import numpy as np

import concourse.bass as bass
import concourse.bacc as bacc
import concourse.tile as tile
from concourse import mybir
from concourse.bass_utils import run_bass_kernel_spmd

B = 8
N = 2048
NT = N // 128  # 16 partition tiles
LAT = 64
XF = LAT + 1  # latent features + ones column (folds b0 in)
HID = 128
ODIM = 64
N_CORES = 8

F32 = mybir.dt.float32
BF16 = mybir.dt.bfloat16
Act = mybir.ActivationFunctionType


@functools.lru_cache(maxsize=2)
def _build_v2():
    nc = bacc.Bacc(None, target_bir_lowering=False, debug=False)

    ATP_d = nc.declare_dram_parameter("ATP", [4, 128, NT, 512], BF16,
                                      isOutput=False)
    XP_d = nc.declare_dram_parameter("XP", [128, NT * XF], BF16,
                                     isOutput=False)
    W0P_d = nc.declare_dram_parameter("W0P", [XF, HID], BF16, isOutput=False)
    W1_d = nc.declare_dram_parameter("W1", [HID, HID], BF16, isOutput=False)
    W2_d = nc.declare_dram_parameter("W2", [HID, HID], BF16, isOutput=False)
    WO_d = nc.declare_dram_parameter("WO", [HID, ODIM], BF16, isOutput=False)
    B1_d = nc.declare_dram_parameter("B1", [128, HID], F32, isOutput=False)
    B2_d = nc.declare_dram_parameter("B2", [128, HID], F32, isOutput=False)
    BO_d = nc.declare_dram_parameter("BO", [ODIM, 1], F32, isOutput=False)
    YT_d = nc.declare_dram_parameter("YT", [ODIM, N], BF16, isOutput=True)

    with tile.TileContext(nc) as tc:
        with (
            tc.tile_pool(name="const", bufs=1) as constp,
            tc.tile_pool(name="at", bufs=1) as atp,
            tc.tile_pool(name="ht", bufs=2) as htp,
            tc.tile_pool(name="msg", bufs=2) as msgp,
            tc.tile_pool(name="msgt", bufs=2) as msgtp,
            tc.tile_pool(name="aht", bufs=2) as ahtsp,
            tc.tile_pool(name="ahtps", bufs=1, space=bass.MemorySpace.PSUM) as ahtpp,
            tc.tile_pool(name="aggp", bufs=4, space=bass.MemorySpace.PSUM) as aggp,
            tc.tile_pool(name="workp", bufs=3, space=bass.MemorySpace.PSUM) as workp,
        ):
            # ---- early-needed constants on the scalar queue; late ones
            # on the sync queue ----
            xp_t = constp.tile([128, NT * XF], BF16, tag="xp")
            nc.scalar.dma_start(xp_t[:], XP_d[:])
            w0p_t = constp.tile([XF, HID], BF16, tag="w0p")
            nc.scalar.dma_start(w0p_t[:], W0P_d[:])
            w1_t = constp.tile([HID, HID], BF16, tag="w1")
            nc.scalar.dma_start(w1_t[:], W1_d[:])
            b1_t = constp.tile([128, HID], F32, tag="b1")
            nc.scalar.dma_start(b1_t[:], B1_d[:])
            w2_t = constp.tile([HID, HID], BF16, tag="w2")
            nc.sync.dma_start(w2_t[:], W2_d[:])
            b2_t = constp.tile([128, HID], F32, tag="b2")
            nc.sync.dma_start(b2_t[:], B2_d[:])
            wo_t = constp.tile([HID, ODIM], BF16, tag="wo")
            nc.sync.dma_start(wo_t[:], WO_d[:])
            bo_t = constp.tile([ODIM, 1], F32, tag="bo")
            nc.sync.dma_start(bo_t[:], BO_d[:])

            # ---- the A^T panel stream: one queue in strict panel order
            # (a second queue would steal HBM bandwidth from panel 0 and
            # delay the pipeline start) ----
            at_t = atp.tile([128, 4 * NT * 512], BF16, tag="at")
            at5 = at_t[:].rearrange("p (i j c) -> p i j c", j=NT, c=512)
            for i in range(4):
                step = 4 if i == 0 else 8
                for j0 in range(0, NT, step):
                    nc.gpsimd.dma_start(
                        at5[:, i, j0 : j0 + step, :],
                        ATP_d[i, :, j0 : j0 + step, :],
                    )

            xp3 = xp_t[:].rearrange("p (t f) -> p t f", f=XF)
            hT1 = htp.tile([128, N], BF16, tag="ht", name="hT1")
            msg1 = msgp.tile([128, N], BF16, tag="msg", name="msg1")
            Alu = mybir.AluOpType

            def agg_mm(ap_ps, i, j, msg_nat, start, stop):
                nc.tensor.matmul(
                    ap_ps[0:HID, :],
                    msg_nat[:, j * 128 : (j + 1) * 128],
                    at5[:, i, j, :],
                    start=start,
                    stop=stop,
                )

            def emit_relu(l, i, ap_ps, hT_next):
                with nc.named_scope(f"relu{l}"):
                    dst = hT_next[:, i * 512 : (i + 1) * 512]
                    if i % 2 == 0:
                        nc.scalar.activation(dst, ap_ps[0:HID, :], Act.Relu)
                    else:
                        nc.vector.tensor_scalar_max(dst, ap_ps[0:HID, :], 0.0)

            def emit_msg_mm(lname, i, q, hT, w_t, mp):
                """msg chunk j=4i+q node-major: stationary = hT feature
                slice, moving = W.  No transposes needed."""
                with nc.named_scope(lname):
                    j = 4 * i + q
                    nc.tensor.matmul(
                        mp[:, q * 128 : (q + 1) * 128],
                        hT[:, j * 128 : (j + 1) * 128],
                        w_t[:],
                        start=True,
                        stop=True,
                    )

            def emit_msg_evac(lname, i, q, b_bc, mp, msg_nat):
                with nc.named_scope(lname):
                    j = 4 * i + q
                    eng = nc.vector
                    eng.scalar_tensor_tensor(
                        msg_nat[:, j * 128 : (j + 1) * 128],
                        mp[:, q * 128 : (q + 1) * 128],
                        1.0,
                        b_bc[:],
                        Alu.mult,
                        Alu.add,
                    )

            def emit_projT(i, hT, eng=None):
                """proj chunk i stays feature-major: mm + per-partition
                bias evac to bf16, then the Y^T slice DMAs out.  The host
                does the final transpose + node-mask multiply."""
                with nc.named_scope("proj"):
                    pp = workp.tile([128, 512], F32, tag="workp",
                                    name=f"pp{i}")
                    nc.tensor.matmul(
                        pp[0:ODIM, :],
                        wo_t[:],
                        hT[:, i * 512 : (i + 1) * 512],
                        start=True,
                        stop=True,
                    )
                    pt = msgtp.tile([128, 512], BF16, tag="msgt",
                                    name=f"projT{i}")
                    if i % 2 == 0:
                        nc.scalar.activation(
                            pt[0:ODIM, :], pp[0:ODIM, :], Act.Identity,
                            bias=bo_t[:],
                        )
                    else:
                        nc.vector.tensor_scalar_add(
                            pt[0:ODIM, :], pp[0:ODIM, :], bo_t[:]
                        )
                    (eng or nc.sync).dma_start(
                        YT_d[:, i * 512 : (i + 1) * 512], pt[0:ODIM, :]
                    )

            # ---- layer 0 chasing the stream + layer-1 partials.
            # agg1 (ip, j) steps fill the PE's dependency waits. ----
            aps1 = {}
            done1 = {ip: set() for ip in range(4)}
            pending = []  # available agg1 (ip, j) steps not yet emitted

            def agg1_step(ip, j):
                if ip not in aps1:
                    aps1[ip] = aggp.tile(
                        [128, 512], F32, tag="agg", name=f"agg1_{ip}"
                    )
                start = not done1[ip]
                done1[ip].add(j)
                agg_mm(aps1[ip], ip, j, msg1,
                       start=start, stop=(len(done1[ip]) == NT))

            def emit_partials(n, max_ip):
                with nc.named_scope("agg1"):
                    k, emitted = 0, 0
                    while k < len(pending) and emitted < n:
                        ip, j = pending[k]
                        if ip <= max_ip:
                            pending.pop(k)
                            agg1_step(ip, j)
                            emitted += 1
                        else:
                            k += 1

            msg2 = None
            hT2 = None
            for i in range(4):
                # everything runnable without panel i goes BEFORE chunk
                # i's aggs: the PE queue is in-order, so work emitted
                # after them would stall behind the panel-i DMA wait
                emit_partials(99, i - 1)
                with nc.named_scope("agg0"):
                    aht_ps = ahtpp.tile([128, 512], F32, tag="ahtps",
                                        name=f"aht{i}")
                    for j in range(NT):
                        nc.tensor.matmul(
                            aht_ps[0:XF, :],
                            xp3[:, j, :],
                            at5[:, i, j, :],
                            start=(j == 0),
                            stop=(j == NT - 1),
                        )
                    aht_sb = ahtsp.tile([128, 512], BF16, tag="aht",
                                        name=f"ahts{i}")
                    if i % 2 == 0:
                        nc.scalar.activation(
                            aht_sb[0:XF, :], aht_ps[0:XF, :], Act.Copy
                        )
                    else:
                        nc.vector.tensor_copy(aht_sb[0:XF, :], aht_ps[0:XF, :])
                emit_partials(4, i)
                with nc.named_scope("post0"):
                    post_ps = workp.tile([128, 512], F32, tag="workp",
                                         name=f"post{i}")
                    nc.tensor.matmul(
                        post_ps[0:HID, :],
                        w0p_t[:],
                        aht_sb[0:XF, :],
                        start=True,
                        stop=True,
                    )
                emit_partials(3, i)
                emit_relu(0, i, post_ps, hT1)
                emit_partials(3, i)
                m1ps = workp.tile([128, 512], F32, tag="workp",
                                  name=f"m1ps{i}")
                for q in range(4):
                    emit_msg_mm("msg1", i, q, hT1, w1_t, m1ps)
                    emit_msg_evac("msg1", i, q, b1_t, m1ps, msg1)
                    if q == 1:
                        emit_partials(2, i)
                for j in range(4 * i, 4 * i + 4):
                    for ip in range(4):
                        pending.append((ip, j))
                if i == 3:
                    # final panel: software-pipelined — chunk ip+1's
                    # remaining agg matmuls cover chunk ip's ReLU, whose
                    # msg2 chunks follow
                    msg2 = msgp.tile([128, N], BF16, tag="msg", name="msg2")
                    hT2 = htp.tile([128, N], BF16, tag="ht", name="hT2")

                    def drain1(ip):
                        with nc.named_scope("agg1"):
                            for (ipp, j) in [p for p in pending
                                             if p[0] == ip]:
                                pending.remove((ipp, j))
                                agg1_step(ip, j)

                    def msg2_chunk(ip):
                        m2ps = workp.tile([128, 512], F32, tag="workp",
                                          name=f"m2ps{ip}")
                        for q in range(4):
                            emit_msg_mm("msg2", ip, q, hT2, w2_t, m2ps)
                            emit_msg_evac("msg2", ip, q, b2_t, m2ps, msg2)

                    drain1(0)
                    emit_relu(1, 0, aps1[0], hT2)
                    drain1(1)
                    msg2_chunk(0)
                    emit_relu(1, 1, aps1[1], hT2)
                    drain1(2)
                    msg2_chunk(1)
                    emit_relu(1, 2, aps1[2], hT2)
                    drain1(3)
                    msg2_chunk(2)
                    emit_relu(1, 3, aps1[3], hT2)
                    msg2_chunk(3)

            # ---- layer 2 + projection; final chunk in 256-halves so
            # the trailing serial chain is short ----
            hT3 = htp.tile([128, N], BF16, tag="ht", name="hT3")
            for i in range(4):
                ap_ps = aggp.tile([128, 512], F32, tag="agg",
                                  name=f"agg2_{i}")
                with nc.named_scope("agg2"):
                    for j in range(NT):
                        agg_mm(ap_ps, i, j, msg2,
                               start=(j == 0), stop=(j == NT - 1))
                if i < 3:
                    emit_relu(2, i, ap_ps, hT3)
                    if i >= 1:
                        emit_projT(i - 1, hT3,
                                   nc.sync if i == 1 else nc.scalar)
            with nc.named_scope("relu2"):
                nc.scalar.activation(
                    hT3[:, 1536:1792], ap_ps[0:HID, 0:256], Act.Relu
                )
                nc.vector.tensor_scalar_max(
                    hT3[:, 1792:2048], ap_ps[0:HID, 256:512], 0.0
                )
            emit_projT(2, hT3, nc.sync)
            with nc.named_scope("proj"):
                pt3 = msgtp.tile([128, 512], BF16, tag="msgt",
                                 name="projT3")
                for h in range(2):
                    c0 = 1536 + h * 256
                    pp = workp.tile([128, 512], F32, tag="workp",
                                    name=f"pp3h{h}")
                    nc.tensor.matmul(
                        pp[0:ODIM, 0:256],
                        wo_t[:],
                        hT3[:, c0 : c0 + 256],
                        start=True,
                        stop=True,
                    )
                    if h == 0:
                        nc.scalar.activation(
                            pt3[0:ODIM, 0:256], pp[0:ODIM, 0:256],
                            Act.Identity, bias=bo_t[:],
                        )
                        nc.scalar.dma_start(
                            YT_d[:, 1536:1792], pt3[0:ODIM, 0:256]
                        )
                    else:
                        nc.vector.tensor_scalar_add(
                            pt3[0:ODIM, 256:512], pp[0:ODIM, 0:256],
                            bo_t[:],
                        )
                        nc.gpsimd.dma_start(
                            YT_d[:, 1792:2048], pt3[0:ODIM, 256:512]
                        )

    nc.compile()
    return nc


def _prep_v2(latent_features, adjacency_matrix, node_mask,
             W0, b0, W1, b1, W2, b2, Wout, bout):
    import ml_dtypes

    bf = ml_dtypes.bfloat16
    lat = np.asarray(latent_features, dtype=np.float32)
    adj = np.asarray(adjacency_matrix, dtype=np.float32)
    # ATP[b, i, p, j, c] = A^T[j*128+p, i*512+c] = A[i*512+c, j*128+p]
    atp = np.ascontiguousarray(
        adj.reshape(B, 4, 512, NT, 128).transpose(0, 1, 4, 3, 2).astype(bf)
    )
    # XP[b, p, t*65+f] = [X|1][t*128+p, f]
    xa = np.concatenate(
        [lat, np.ones((B, N, 1), np.float32)], axis=2
    )  # [B, N, 65]
    xp = np.ascontiguousarray(
        xa.reshape(B, NT, 128, XF).transpose(0, 2, 1, 3)
        .reshape(B, 128, NT * XF).astype(bf)
    )
    w0p = np.ascontiguousarray(
        np.concatenate(
            [np.asarray(W0, np.float32),
             np.asarray(b0, np.float32).reshape(1, HID)],
            axis=0,
        ).astype(bf)
    )
    w1b = np.ascontiguousarray(np.asarray(W1, np.float32).astype(bf))
    w2b = np.ascontiguousarray(np.asarray(W2, np.float32).astype(bf))
    wob = np.ascontiguousarray(np.asarray(Wout, np.float32).astype(bf))
    b1_ = np.ascontiguousarray(
        np.broadcast_to(np.asarray(b1, np.float32).reshape(1, HID),
                        (128, HID))
    )
    b2_ = np.ascontiguousarray(
        np.broadcast_to(np.asarray(b2, np.float32).reshape(1, HID),
                        (128, HID))
    )
    bo_ = np.asarray(bout, np.float32).reshape(ODIM, 1)

    in_maps = []
    for c in range(N_CORES):
        in_maps.append(
            {
                "ATP": atp[c],
                "XP": xp[c],
                "W0P": w0p,
                "W1": w1b,
                "B1": b1_,
                "W2": w2b,
                "B2": b2_,
                "WO": wob,
                "BO": bo_,
            }
        )
    return in_maps


def kernel(
    latent_features,
    adjacency_matrix,
    node_mask,
    W0,
    b0,
    W1,
    b1,
    W2,
    b2,
    Wout,
    bout,
    _trace=False,
    _agg_dt=None,
):
    nc = _build_v2()
    in_maps = _prep_v2(latent_features, adjacency_matrix, node_mask,
                       W0, b0, W1, b1, W2, b2, Wout, bout)
    res = run_bass_kernel_spmd(
        nc, in_maps, core_ids=list(range(N_CORES)), trace=_trace
    )
    msk = np.asarray(node_mask, dtype=np.float32)  # [B, N, 1]
    out = np.stack(
        [
            np.asarray(res.results[c]["YT"]).astype(np.float32).T
            for c in range(N_CORES)
        ],
        axis=0,
    ) * msk
    if _trace:
        return out, res
    return out



# revision 54
# speedup vs baseline: 1.5388x; 1.5388x over previous
"""Trainium2 Bass kernel for a 3-layer dense-adjacency GCN decoder (v8, fp8).

Problem (per batch graph): 3x GCN layer (msg = h@W + b; agg = A @ msg; relu)
followed by output projection + node mask. B=8 graphs of N=2048 nodes,
latent=64, hidden=128, out=64. Batch-parallel: one graph per NeuronCore.

v13 plan — fp8 DoubleRow aggregation (measured ~52 us vs the bf16 v7's
~76-82 us; rel err 3.1e-3 vs 3.9e-3):
  - A is zero-centered and quantized host-side: ATP = fp8e4(A - 0.5) in the
    v7 panel layout (ATP[i][p][j][c] = A~[i*512+c, j*128+p], 4 MB — half
    the bf16 DMA). The mean term 0.5*1(x)colsum(msg) is restored EXACTLY
    via per-partition [128,1] bias vectors at every ReLU evacuation, so the
    dominant mean-cascade of the network carries no quantization error;
    only the zero-mean residual sees fp8 noise.
  - All agg matmuls are fp8 DoubleRow: K=256 j-pairs, 1024-wide fp8 moving
    (measured 216 ns/MM warm = 2x bf16 work rate). The doubled DR
    LDWEIGHTS (no FWL) is amortized j-outer/i-inner: one msg-pair load
    feeds 4 panel matmuls (layer1) / a bank pair (layer2).
  - Layer 0 is the reassociated A-pass on [X|1] (panel-major, chases the
    DMA stream): stationary = fp8 [X|1|pad] j-pairs (XFP=80 for the
    16B-aligned pair stride), bias c0 = 0.5*colsum([X|1])@W0P host-computed
    from the EXACT f32 X.
  - msg tiles are single fp8, power-of-2 scaled (s1=1, s2=2^-9; exact).
    Scales fold into host-preprocessed weights via ReLU homogeneity; proj
    unfolds via Wout/s2. Biases (zero here, handled generally) ride the
    msg evacuation + colsum-correction constants.
  - colsum(msg_l) is computed on device exactly: accum_out on the ReLU
    evacuations gives hsum = colsum(hT), then a [128,1] matmul with the
    f32 W copy gives the next layer's ReLU bias vector.
  - The A stream rides the gpsimd/SWDGE queue only (HWDGE measures ~4x
    slower here and shares the same SDMA engines): panel 0 in quarters,
    panels 1-2 in halves, panel 3 tapered 8+6+2 so the trailing
    xpass/relu0/msg1 chain starts before the last sliver lands.
  - HAM keep-warm: 8 junk matmuls on a zeroed tile cover the framework
    preamble; zero-contribution filler matmuls (accumulate-0 into the live
    PSUM bank) pad chunk-wait gaps so the PE clock-gate stays at 2.4 GHz.
  - Layer-2 aggs run in bank pairs: banks 0,1 finish first and their
    relu2/proj/output DMA overlap banks 2,3's aggregation. Proj results
    collect in one SBUF tile and ship as two 128 KB DMAs (per-chunk SWDGE
    descriptor-gen was ~0.65 us each at the tail).
  - Output stays feature-major bf16 Y^T; host does transpose + node-mask +
    f32 upcast.
"""

import functools

import numpy as np

import concourse.bass as bass
import concourse.bacc as bacc
import concourse.tile as tile
from concourse import mybir
from concourse.bass_utils import run_bass_kernel_spmd

B = 8
N = 2048
NT = N // 128   # 16 contraction chunks
NP = NT // 2    # 8 DoubleRow j-pairs
LAT = 64
XF = LAT + 1    # latent + ones column (folds b0)
XFP = 80        # padded to 16-elem alignment for the DoubleRow pair stride
HID = 128
ODIM = 64
N_CORES = 8
NWARM = 7

S1 = 1.0        # msg1 fp8 scale (power of 2 -> exact)
S2 = 2.0 ** -9  # msg2 fp8 scale

F32 = mybir.dt.float32
BF16 = mybir.dt.bfloat16
FP8 = mybir.dt.float8e4
Act = mybir.ActivationFunctionType
Alu = mybir.AluOpType
DR = mybir.MatmulPerfMode.DoubleRow


@functools.lru_cache(maxsize=2)
def _build_v8(zero_bias=True):
    nc = bacc.Bacc(None, target_bir_lowering=False, debug=False)

    ATP_d = nc.declare_dram_parameter("ATP", [4, 128, NT, 512], FP8,
                                      isOutput=False)
    XP_d = nc.declare_dram_parameter("XP", [128, NT * XFP], FP8,
                                     isOutput=False)
    W0P_d = nc.declare_dram_parameter("W0P", [XFP, HID], BF16, isOutput=False)
    C0_d = nc.declare_dram_parameter("C0", [HID, 1], F32, isOutput=False)
    W1S_d = nc.declare_dram_parameter("W1S", [HID, HID], BF16, isOutput=False)
    W1SF_d = nc.declare_dram_parameter("W1SF", [HID, HID], F32,
                                       isOutput=False)
    SB1_d = nc.declare_dram_parameter("SB1", [128, HID], F32, isOutput=False)
    SC1_d = nc.declare_dram_parameter("SC1", [HID, 1], F32, isOutput=False)
    W2S_d = nc.declare_dram_parameter("W2S", [HID, HID], BF16, isOutput=False)
    W2SF_d = nc.declare_dram_parameter("W2SF", [HID, HID], F32,
                                       isOutput=False)
    SB2_d = nc.declare_dram_parameter("SB2", [128, HID], F32, isOutput=False)
    SC2_d = nc.declare_dram_parameter("SC2", [HID, 1], F32, isOutput=False)
    WOS_d = nc.declare_dram_parameter("WOS", [HID, ODIM], BF16, isOutput=False)
    BO_d = nc.declare_dram_parameter("BO", [ODIM, 1], F32, isOutput=False)
    YT_d = nc.declare_dram_parameter("YT", [ODIM, N], BF16, isOutput=True)

    with tile.TileContext(nc) as tc:
        with (
            tc.tile_pool(name="const", bufs=1) as constp,
            tc.tile_pool(name="at", bufs=1) as atp,
            tc.tile_pool(name="ht", bufs=2) as htp,
            tc.tile_pool(name="msg", bufs=2) as msgp,
            tc.tile_pool(name="aht", bufs=2) as ahtp,
            tc.tile_pool(name="misc", bufs=1) as miscp,
            tc.tile_pool(name="yt", bufs=2) as ytp,
            tc.tile_pool(name="aggps", bufs=4, space=bass.MemorySpace.PSUM) as aggpp,
            tc.tile_pool(name="mps", bufs=2, space=bass.MemorySpace.PSUM) as mpp,
            tc.tile_pool(name="wps", bufs=2, space=bass.MemorySpace.PSUM) as wpp,
        ):
            # ---- A~ panel stream across THREE queues (gpsimd + the two
            # HWDGE rings) so per-transfer completion latency overlaps and
            # panel 0 lands early. Early consts ride scalar, late on sync
            # behind its A chunks. ----
            at_t = atp.tile([128, 4 * NT * 512], FP8, tag="at")
            at5 = at_t[:].rearrange("p (i j c) -> p i j c", j=NT, c=512)

            def a_chunk(q, i, j0, step):
                q.dma_start(at5[:, i, j0 : j0 + step, :],
                            ATP_d[i, :, j0 : j0 + step, :])

            xp_t = constp.tile([128, NT * XFP], FP8, tag="xp")
            nc.scalar.dma_start(xp_t[:], XP_d[:])
            # All A chunks ride the gpsimd/SWDGE queue: HWDGE shares the
            # same 16 SDMA engines but is ~4x slower per byte here, so
            # offloading to it only hurts. Panel 0 in quarters so the
            # chase starts early; panels 1-3 whole (fewer descriptor gens).
            for j0 in range(0, NT, 4):
                a_chunk(nc.gpsimd, 0, j0, 4)
            a_chunk(nc.gpsimd, 1, 0, 8)
            a_chunk(nc.gpsimd, 1, 8, 8)
            a_chunk(nc.gpsimd, 2, 0, 8)
            a_chunk(nc.gpsimd, 2, 8, 8)
            # panel 3 tapers (8+6+2) so the final xpass/aht/relu0/msg1
            # chain starts before the last sliver lands
            a_chunk(nc.gpsimd, 3, 0, 8)
            a_chunk(nc.gpsimd, 3, 8, 6)
            a_chunk(nc.gpsimd, 3, 14, 2)
            # early consts (needed during the stream) on scalar
            w0p_t = constp.tile([XFP, HID], BF16, tag="w0p")
            nc.scalar.dma_start(w0p_t[:], W0P_d[:])
            c0_t = constp.tile([HID, 1], F32, tag="c0")
            nc.scalar.dma_start(c0_t[:], C0_d[:])
            w1s_t = constp.tile([HID, HID], BF16, tag="w1s")
            nc.scalar.dma_start(w1s_t[:], W1S_d[:])
            sb1_t = constp.tile([128, HID], F32, tag="sb1")
            nc.scalar.dma_start(sb1_t[:], SB1_d[:])
            sc1_t = constp.tile([HID, 1], F32, tag="sc1")
            nc.scalar.dma_start(sc1_t[:], SC1_d[:])
            w1sf_t = constp.tile([HID, HID], F32, tag="w1sf")
            nc.scalar.dma_start(w1sf_t[:], W1SF_d[:])
            # late consts (layer-2/proj) behind sync's A chunks
            w2s_t = constp.tile([HID, HID], BF16, tag="w2s")
            nc.sync.dma_start(w2s_t[:], W2S_d[:])
            w2sf_t = constp.tile([HID, HID], F32, tag="w2sf")
            nc.sync.dma_start(w2sf_t[:], W2SF_d[:])
            sb2_t = constp.tile([128, HID], F32, tag="sb2")
            nc.sync.dma_start(sb2_t[:], SB2_d[:])
            sc2_t = constp.tile([HID, 1], F32, tag="sc2")
            nc.sync.dma_start(sc2_t[:], SC2_d[:])
            wos_t = constp.tile([HID, ODIM], BF16, tag="wos")
            nc.sync.dma_start(wos_t[:], WOS_d[:])
            bo_t = constp.tile([ODIM, 1], F32, tag="bo")
            nc.sync.dma_start(bo_t[:], BO_d[:])

            # ---- PE warm-up on a zeroed tile (HAM un-throttle during the
            # DMA preamble; results never read) ----
            junk_t = constp.tile([128, 512], BF16, tag="junk")
            nc.vector.memset(junk_t[:], 0.0)
            with nc.named_scope("warmup"):
                junk_ps = aggpp.tile([128, 512], F32, tag="agg", name="junkps")
                for _ in range(NWARM):
                    nc.tensor.matmul(junk_ps[:, :], junk_t[:, 0:128],
                                     junk_t[:, :], start=True, stop=True)

            xp4 = xp_t[:].rearrange("p (jp g f) -> p jp g f", jp=NP, g=2,
                                    f=XFP)

            hT1 = htp.tile([128, N], BF16, tag="ht", name="hT1")
            msg1 = msgp.tile([128, N], FP8, tag="msg", name="msg1")
            acc1 = [miscp.tile([128, 1], F32, tag=f"acc1_{i}",
                               name=f"acc1_{i}") for i in range(4)]
            acc3b = miscp.tile([128, 1], F32, tag="acc3b", name="acc3b")

            def evac_relu(lname, i, src_ps, hT_dst, bias_t, acc_t, on_scalar):
                """hT chunk i = relu(psum + bias); accum_out = row-sums."""
                with nc.named_scope(lname):
                    dst = hT_dst[:, i * 512 : (i + 1) * 512]
                    kw = {}
                    if acc_t is not None:
                        kw["accum_out"] = acc_t[:]
                    if on_scalar:
                        nc.scalar.activation(dst, src_ps, Act.Relu,
                                             bias=bias_t[:], **kw)
                    else:
                        # NB: tensor_scalar's scalar2/op1 is silently dropped
                        # (and its accum_out is not a row-sum) — use stt with
                        # a zeros tensor for the max instead.
                        nc.vector.scalar_tensor_tensor(
                            dst, src_ps, bias_t[:], junk_t[:, 0:512],
                            Alu.add, Alu.max, **kw)

            def emit_msg(lname, q, hT, w_t, sb_t, msg_dst, on_scalar):
                """msg chunk q (node-major fp8): prod matmul + bias evac.
                GpSimd cannot read PSUM, so evacs go to DVE; with zero
                biases alternate chunks use the scalar engine (plain cast),
                keeping both engines fed."""
                with nc.named_scope(lname):
                    mp = mpp.tile([128, HID], F32, tag="mp",
                                  name=f"{lname}_{q}")
                    nc.tensor.matmul(mp[:], hT[:, q * 128 : (q + 1) * 128],
                                     w_t[:], start=True, stop=True)
                    dst = msg_dst[:, q * 128 : (q + 1) * 128]
                    if zero_bias and on_scalar:
                        nc.scalar.activation(dst, mp[:], Act.Copy)
                    else:
                        nc.vector.scalar_tensor_tensor(
                            dst, mp[:], 1.0, sb_t[:], Alu.mult, Alu.add,
                        )

            # ---- stream phase: layer-0 A-pass chases the panels; hT1 and
            # msg1 are produced per panel while the next panel streams ----
            def filler(ps, first=False):
                """Zero-contribution matmul into the live accumulation bank:
                keeps the PE busy (HAM warm) while waiting for DMA chunks
                without any pool or numerics side effects."""
                nc.tensor.matmul(ps[0:XFP, :], junk_t[:, 0:XFP],
                                 junk_t[:, :], start=first, stop=False)

            for i in range(4):
                with nc.named_scope("xpass"):
                    aht_ps = aggpp.tile([128, 512], F32, tag="agg",
                                        name=f"ahtps{i}")
                    if i > 0:
                        filler(aht_ps, first=True)
                        filler(aht_ps)
                    for jp in range(NP):
                        nc.tensor.matmul(
                            aht_ps[0:XFP, :],
                            xp4[:, jp, :, :],
                            at5[:, i, 2 * jp : 2 * jp + 2, :],
                            start=(i == 0 and jp == 0),
                            stop=(jp == NP - 1),
                            perf_mode=DR,
                        )
                        if i == 0 and jp in (1, 3, 5):
                            filler(aht_ps)
                    aht_sb = ahtp.tile([128, 512], BF16, tag="aht",
                                       name=f"ahts{i}")
                    if i == 3:
                        # split the post-stream tail chain in 256-col
                        # halves across scalar+vector so the two halves
                        # run concurrently
                        nc.scalar.activation(aht_sb[0:XFP, 0:256],
                                             aht_ps[0:XFP, 0:256], Act.Copy)
                        nc.vector.tensor_copy(aht_sb[0:XFP, 256:512],
                                              aht_ps[0:XFP, 256:512])
                    elif i % 2 == 0:
                        nc.scalar.activation(aht_sb[0:XFP, :],
                                             aht_ps[0:XFP, :], Act.Copy)
                    else:
                        nc.vector.tensor_copy(aht_sb[0:XFP, :],
                                              aht_ps[0:XFP, :])
                with nc.named_scope("post0"):
                    wp = wpp.tile([128, 512], F32, tag="wp", name=f"post{i}")
                    if i == 3:
                        nc.tensor.matmul(wp[0:HID, 0:256], w0p_t[:],
                                         aht_sb[0:XFP, 0:256],
                                         start=True, stop=True)
                        nc.tensor.matmul(wp[0:HID, 256:512], w0p_t[:],
                                         aht_sb[0:XFP, 256:512],
                                         start=True, stop=True)
                    else:
                        nc.tensor.matmul(wp[0:HID, :], w0p_t[:],
                                         aht_sb[0:XFP, :],
                                         start=True, stop=True)
                if i == 3:
                    with nc.named_scope("relu0"):
                        nc.scalar.activation(hT1[:, 1536:1792],
                                             wp[0:HID, 0:256], Act.Relu,
                                             bias=c0_t[:],
                                             accum_out=acc1[3][:])
                        nc.vector.scalar_tensor_tensor(
                            hT1[:, 1792:2048], wp[0:HID, 256:512], c0_t[:],
                            junk_t[:, 0:256], Alu.add, Alu.max,
                            accum_out=acc3b[:])
                else:
                    evac_relu("relu0", i, wp[0:HID, :], hT1, c0_t, acc1[i],
                              on_scalar=(i % 2 == 1))
                for q in range(4 * i, 4 * i + 4):
                    emit_msg("msg1", q, hT1, w1s_t, sb1_t, msg1,
                             on_scalar=(q % 2 == 1))

            # ---- layer-1 aggs, j-outer/i-inner (one msg-pair weight load
            # feeds 4 panel matmuls) ----
            msg1p = msg1[:].rearrange("p (jp g m) -> p jp g m", jp=NP, g=2,
                                      m=128)
            agg1 = [aggpp.tile([128, 512], F32, tag="agg", name=f"agg1_{i}")
                    for i in range(4)]
            svec1 = miscp.tile([HID, 1], F32, tag="svec1", name="svec1")
            for jp in range(NP):
                with nc.named_scope("agg1"):
                    for i in range(4):
                        nc.tensor.matmul(
                            agg1[i][0:HID, :],
                            msg1p[:, jp, :, :],
                            at5[:, i, 2 * jp : 2 * jp + 2, :],
                            start=(jp == 0),
                            stop=(jp == NP - 1),
                            perf_mode=DR,
                        )
                if jp == 1:
                    # colsum correction vector for the relu1 bias:
                    # svec1 = 0.5*(colsum(hT1) @ W1S) + SC1
                    with nc.named_scope("svec1"):
                        t01 = miscp.tile([128, 1], F32, tag="t01")
                        t23 = miscp.tile([128, 1], F32, tag="t23")
                        hsum = miscp.tile([128, 1], F32, tag="hsum1")
                        nc.vector.tensor_add(t01[:], acc1[0][:], acc1[1][:])
                        nc.gpsimd.tensor_add(t23[:], acc1[2][:], acc1[3][:])
                        nc.gpsimd.tensor_add(t23[:], t23[:], acc3b[:])
                        nc.vector.tensor_add(hsum[:], t01[:], t23[:])
                        sp = wpp.tile([128, 512], F32, tag="wp", name="smm1")
                        nc.tensor.matmul(sp[0:HID, 0:1], w1sf_t[:], hsum[:],
                                         start=True, stop=True)
                        nc.vector.scalar_tensor_tensor(
                            svec1[:], sp[0:HID, 0:1], 0.5, sc1_t[:],
                            Alu.mult, Alu.add,
                        )

            # ---- relu1 -> hT2 -> msg2, interleaved with layer-2 aggs ----
            hT2 = htp.tile([128, N], BF16, tag="ht", name="hT2")
            msg2 = msgp.tile([128, N], FP8, tag="msg", name="msg2")
            acc2 = [miscp.tile([128, 1], F32, tag=f"acc2_{i}",
                               name=f"acc2_{i}") for i in range(4)]
            msg2p = msg2[:].rearrange("p (jp g m) -> p jp g m", jp=NP, g=2,
                                      m=128)
            agg2 = [aggpp.tile([128, 512], F32, tag="agg", name=f"agg2_{i}")
                    for i in range(4)]
            svec2 = miscp.tile([HID, 1], F32, tag="svec2", name="svec2")

            def agg2_pair(jp, banks):
                with nc.named_scope("agg2"):
                    for i in banks:
                        nc.tensor.matmul(
                            agg2[i][0:HID, :],
                            msg2p[:, jp, :, :],
                            at5[:, i, 2 * jp : 2 * jp + 2, :],
                            start=(jp == 0),
                            stop=(jp == NP - 1),
                            perf_mode=DR,
                        )

            # paired relu1 evacs run on scalar+vector concurrently
            evac_relu("relu1", 0, agg1[0][0:HID, :], hT2, svec1, acc2[0],
                      on_scalar=True)
            evac_relu("relu1", 1, agg1[1][0:HID, :], hT2, svec1, acc2[1],
                      on_scalar=False)
            for q in range(0, 8):
                emit_msg("msg2", q, hT2, w2s_t, sb2_t, msg2,
                         on_scalar=(q % 2 == 1))
            agg2_pair(0, (0, 1))
            agg2_pair(1, (0, 1))
            evac_relu("relu1", 2, agg1[2][0:HID, :], hT2, svec1, acc2[2],
                      on_scalar=True)
            evac_relu("relu1", 3, agg1[3][0:HID, :], hT2, svec1, acc2[3],
                      on_scalar=False)
            for q in range(8, 16):
                emit_msg("msg2", q, hT2, w2s_t, sb2_t, msg2,
                         on_scalar=(q % 2 == 1))
            agg2_pair(2, (0, 1))
            agg2_pair(3, (0, 1))
            with nc.named_scope("svec2"):
                t01b = miscp.tile([128, 1], F32, tag="t01b")
                t23b = miscp.tile([128, 1], F32, tag="t23b")
                hsum2 = miscp.tile([128, 1], F32, tag="hsum2")
                nc.vector.tensor_add(t01b[:], acc2[0][:], acc2[1][:])
                nc.gpsimd.tensor_add(t23b[:], acc2[2][:], acc2[3][:])
                nc.vector.tensor_add(hsum2[:], t01b[:], t23b[:])
                sp2 = wpp.tile([128, 512], F32, tag="wp", name="smm2")
                nc.tensor.matmul(sp2[0:HID, 0:1], w2sf_t[:], hsum2[:],
                                 start=True, stop=True)
                nc.vector.scalar_tensor_tensor(
                    svec2[:], sp2[0:HID, 0:1], 0.5, sc2_t[:],
                    Alu.mult, Alu.add,
                )
            for jp in range(4, NP):
                agg2_pair(jp, (0, 1))

            # ---- relu2 -> hT3 -> projection -> Y^T out; final chunk in
            # 256-halves so the trailing serial chain is short ----
            hT3 = htp.tile([128, N], BF16, tag="ht", name="hT3")

            ytall = ytp.tile([128, N], BF16, tag="yt", name="ytall")

            def emit_proj(i, eng_dma=None):
                with nc.named_scope("proj"):
                    pp = wpp.tile([128, 512], F32, tag="wp", name=f"pp{i}")
                    nc.tensor.matmul(pp[0:ODIM, :], wos_t[:],
                                     hT3[:, i * 512 : (i + 1) * 512],
                                     start=True, stop=True)
                    pt = ytall[0:ODIM, i * 512 : (i + 1) * 512]
                    if i % 2 == 0:
                        nc.scalar.activation(pt, pp[0:ODIM, :],
                                             Act.Identity, bias=bo_t[:])
                    else:
                        nc.vector.tensor_scalar_add(pt, pp[0:ODIM, :],
                                                    bo_t[:])

            # banks 0,1 evacuate + project while banks 2,3 aggregate;
            # all Y^T output DMAs ride the fast gpsimd/SWDGE queue
            evac_relu("relu2", 0, agg2[0][0:HID, :], hT3, svec2, None,
                      on_scalar=True)
            evac_relu("relu2", 1, agg2[1][0:HID, :], hT3, svec2, None,
                      on_scalar=False)
            emit_proj(0)
            emit_proj(1)
            nc.gpsimd.dma_start(YT_d[:, 0:1024], ytall[0:ODIM, 0:1024])
            for jp in range(NP):
                agg2_pair(jp, (2, 3))
            # final two banks: relu evacs fan out over scalar+vector
            # concurrently, then the projs chain tightly
            with nc.named_scope("relu2"):
                nc.scalar.activation(hT3[:, 1024:1536],
                                     agg2[2][0:HID, :], Act.Relu,
                                     bias=svec2[:])
                nc.vector.scalar_tensor_tensor(
                    hT3[:, 1536:1792], agg2[3][0:HID, 0:256], svec2[:],
                    junk_t[:, 0:256], Alu.add, Alu.max)
                nc.vector.scalar_tensor_tensor(
                    hT3[:, 1792:2048], agg2[3][0:HID, 256:512], svec2[:],
                    junk_t[:, 0:256], Alu.add, Alu.max)
            emit_proj(2)
            nc.gpsimd.dma_start(YT_d[:, 1024:1536], ytall[0:ODIM, 1024:1536])
            with nc.named_scope("proj"):
                for h in range(2):
                    c0h = 1536 + h * 256
                    pp3 = wpp.tile([128, 512], F32, tag="wp", name=f"pp3h{h}")
                    nc.tensor.matmul(pp3[0:ODIM, 0:256], wos_t[:],
                                     hT3[:, c0h : c0h + 256],
                                     start=True, stop=True)
                    if h == 0:
                        nc.scalar.activation(ytall[0:ODIM, 1536:1792],
                                             pp3[0:ODIM, 0:256],
                                             Act.Identity, bias=bo_t[:])
                    else:
                        nc.vector.tensor_scalar_add(
                            ytall[0:ODIM, 1792:2048],
                            pp3[0:ODIM, 0:256], bo_t[:])
            nc.gpsimd.dma_start(YT_d[:, 1536:2048], ytall[0:ODIM, 1536:2048])

    nc.compile()
    return nc


def _prep_v8(latent_features, adjacency_matrix, node_mask,
             W0, b0, W1, b1, W2, b2, Wout, bout):
    import ml_dtypes

    f8 = ml_dtypes.float8_e4m3
    bf = ml_dtypes.bfloat16
    lat = np.asarray(latent_features, dtype=np.float32)
    adj = np.asarray(adjacency_matrix, dtype=np.float32)
    # ATP[b, i, p, j, c] = (A - 0.5)[i*512+c, j*128+p]
    atp = np.ascontiguousarray(
        np.clip(adj - 0.5, -240.0, 240.0)
        .reshape(B, 4, 512, NT, 128).transpose(0, 1, 4, 3, 2).astype(f8)
    )
    # XP[b, p, t*80+f] = [X|1|pad][t*128+p, f]
    xa = np.concatenate(
        [lat, np.ones((B, N, 1), np.float32),
         np.zeros((B, N, XFP - XF), np.float32)], axis=2
    )  # [B, N, 80]
    xp = np.ascontiguousarray(
        np.clip(xa, -240.0, 240.0)
        .reshape(B, NT, 128, XFP).transpose(0, 2, 1, 3)
        .reshape(B, 128, NT * XFP).astype(f8)
    )
    w0p_f = np.concatenate(
        [np.asarray(W0, np.float32),
         np.asarray(b0, np.float32).reshape(1, HID),
         np.zeros((XFP - XF, HID), np.float32)], axis=0
    )  # [80, HID]
    w0p = np.ascontiguousarray(w0p_f.astype(bf))
    # c0 = 0.5 * colsum([X|1|pad]) @ W0P  (exact f32 colsums, device W0P)
    cs = xa.sum(axis=1)                       # [B, 80]
    c0 = 0.5 * cs @ w0p.astype(np.float32)    # [B, HID]
    c0 = np.ascontiguousarray(c0.reshape(B, HID, 1))

    w1s = np.ascontiguousarray((S1 * np.asarray(W1, np.float32)).astype(bf))
    w1sf = np.ascontiguousarray(S1 * np.asarray(W1, np.float32))
    w2s = np.ascontiguousarray(
        ((S2 / S1) * np.asarray(W2, np.float32)).astype(bf))
    w2sf = np.ascontiguousarray((S2 / S1) * np.asarray(W2, np.float32))
    wos = np.ascontiguousarray(
        (np.asarray(Wout, np.float32) / S2).astype(bf))
    sb1 = np.ascontiguousarray(np.broadcast_to(
        S1 * np.asarray(b1, np.float32).reshape(1, HID), (128, HID)))
    sb2 = np.ascontiguousarray(np.broadcast_to(
        S2 * np.asarray(b2, np.float32).reshape(1, HID), (128, HID)))
    sc1 = np.ascontiguousarray(
        (0.5 * N * S1 * np.asarray(b1, np.float32)).reshape(HID, 1))
    sc2 = np.ascontiguousarray(
        (0.5 * N * S2 * np.asarray(b2, np.float32)).reshape(HID, 1))
    bo_ = np.asarray(bout, np.float32).reshape(ODIM, 1)

    in_maps = []
    for c in range(N_CORES):
        in_maps.append(
            {
                "ATP": atp[c],
                "XP": xp[c],
                "W0P": w0p,
                "C0": c0[c],
                "W1S": w1s,
                "W1SF": w1sf,
                "SB1": sb1,
                "SC1": sc1,
                "W2S": w2s,
                "W2SF": w2sf,
                "SB2": sb2,
                "SC2": sc2,
                "WOS": wos,
                "BO": bo_,
            }
        )
    return in_maps


def kernel(
    latent_features,
    adjacency_matrix,
    node_mask,
    W0,
    b0,
    W1,
    b1,
    W2,
    b2,
    Wout,
    bout,
    _trace=False,
    _agg_dt=None,
):
    zb = all(
        float(np.abs(np.asarray(v, np.float32)).max()) == 0.0
        for v in (b1, b2)
    )
    nc = _build_v8(zero_bias=zb)
    in_maps = _prep_v8(latent_features, adjacency_matrix, node_mask,
                       W0, b0, W1, b1, W2, b2, Wout, bout)
    res = run_bass_kernel_spmd(
        nc, in_maps, core_ids=list(range(N_CORES)), trace=_trace
    )
    msk = np.asarray(node_mask, dtype=np.float32)  # [B, N, 1]
    out = np.stack(
        [
            np.asarray(res.results[c]["YT"]).astype(np.float32).T
            for c in range(N_CORES)
        ],
        axis=0,
    ) * msk
    if _trace:
        return out, res
    return out
